# revision 37
# baseline (speedup 1.0000x reference)
"""DeBut-factorized 1D conv (kernel_size=4) on 8 Trainium2 NeuronCores.

Math: y[b,:,l] = W @ im2col_row(b,l) + bias, where W (512x2048) is a chain of
4 block-diagonal butterfly factors T4@T3@T2@T1.  We fold T3@T2@T1 into stage A
(block-diagonal, 8 dense [128 out x 256 in] blocks) and keep T4 (an 8x16 grid
of 64-wide diagonals) as stage B.

Default mode "v2" (rotated layout, ~45us/exec steady state, ~6.7e-4 rel err):
  * stage A: per 512-column l-tile, 16 waves of 4 CONCURRENT M=32 matmuls on
    the four PE column groups (tile_position=(0, qp*32), qp=(q+Dp)%4 rotation
    so each wave's tiles hit distinct groups).  A wave costs one N=512 stream
    (~216ns) - the col-tiled matmuls run concurrently (measured dstart~10ns).
    The M=32 splitting lands u in a (Dp, P) d-major-tile layout...
  * ...which collapses stage B (T4) to 8 M=128 matmuls per tile: y-tile Dp
    contracts just u tiles (Dp, P=0..1).  24 wave-equivalents/tile total vs
    48 for the natural layout: PE floor 24*512cy = 41.5us/core @2.4GHz.
  * im2col is free: stage-A rhs for (window offset i, channel block) is a
    shifted free-dim slice of a resident [128, 2048] x slab.
  * x is cast to fp16 on the host (halves load bytes; 4KB descriptors); y is
    stored as fp16 and upcast on the host (stores only run on 8 of 16 SDMA
    engines, so fp32 stores at 8.4MB/core would floor at ~47us > PE floor).
  * psum->sbuf traffic balanced DVE/ACT (4+2 ops each per tile); y stores
    split across the sync and scalar DGE rings; PE warm-up burst unthrottles
    the HAM clock gate during the initial slab loads.

Sharding: data-parallel over batch - each of the 8 cores takes 2 of the 16
batches; twiddle/bias-derived weights are replicated (host-precomputed).
"""

import os
import numpy as np

# ---------------------------------------------------------------- constants
R_SHAPES = [[2048, 2048, 4, 4, 1],
            [2048, 2048, 8, 8, 4],
            [1024, 2048, 4, 8, 32],
            [512, 1024, 8, 16, 64]]
KW = 4              # conv kernel size
B_FULL, C_IN, L_IN = 16, 512, 2048
L_OUT = L_IN - (KW - 1)          # 2045
N_CORES = 8
B_SH = B_FULL // N_CORES         # 2 batches per core
NT = 512                         # l-tile width (PSUM bank = 512 f32)
# last tile overlaps by 3 columns (recomputed identically) so every tile is a
# uniform 512 wide — fp32r matmuls require an even free dim, and 512 keeps the
# window reads inside x's 2048 columns.
L_TILES = [(0, 512), (512, 512), (1024, 512), (1533, 512)]

# "v2":  rotated layout exploiting PE col-group concurrency (16 4-wide
#        stage-A waves + 8 stage-B matmuls per tile = 24 wave-equivalents
#        vs natural's 48), x host-cast to f16 + full-row slab DMA loads,
#        y accumulated in row-wide SBUF tiles for 8KB-descriptor stores.
# "f16nat": fp16 inputs, fp32 accumulate, natural layout (~83us measured)
# "f32r": reduced-precision full-rate fp32 PE path (natural layout, ~3e-4)
# "f32" : exact fp32, 4 cyc/row (rotated layout)
# "bf16"/"f16": 16-bit rotated col-tiled layouts (per-l-tile x DMA; slower)
MM_MODE = os.environ.get("DEBUT_MM_MODE", "v2")

_CACHE = {}          # mode -> compiled Bacc module
LAST_RESULT = None   # BassKernelResults of the most recent run (for test.py)


# ------------------------------------------------------- host-side weights
def _build_T_matrices(twiddle):
    Ts, p = [], 0
    for (osz, isz, row, col, diag) in R_SHAPES:
        g = isz // (col * diag)
        n_p = col * osz
        t = twiddle[p:p + n_p].reshape(g, diag, row, col).transpose(0, 2, 3, 1)
        T = np.zeros((osz, isz), dtype=np.float64)
        gi = np.arange(g)[:, None, None, None]
        ri = np.arange(row)[None, :, None, None]
        ci = np.arange(col)[None, None, :, None]
        di = np.arange(diag)[None, None, None, :]
        oi = np.broadcast_to((gi * row + ri) * diag + di, t.shape)
        ii = np.broadcast_to((gi * col + ci) * diag + di, t.shape)
        T[oi.ravel(), ii.ravel()] = t.ravel()
        Ts.append(T)
        p += n_p
    return Ts


def _make_weights(twiddle, bias, rotated):
    """Returns wA [16,128,128], wB list, bias_t [128,4] (f32).

    natural: wB has 32 tiles (h*8 + g3), y-tile h = features [128h, 128h+128).
    rotated: wB has 8 tiles (Dp*2 + P), y-tile Dp features r*64 + 16*Dp + j
             at partition m = r*16 + j.
    """
    T1, T2, T3, T4 = _build_T_matrices(twiddle.astype(np.float64))
    M321 = T3 @ T2 @ T1                       # block-diag: 8 x [128 out, 256 in]

    m = np.arange(128)
    if rotated:
        # stage-A M column m = Dp*32 + j*2 + b -> block-local feature b*64+16Dp+j
        Dp_of_m, s = m // 32, m % 32
        o_of_m = (s % 2) * 64 + 16 * Dp_of_m + (s // 2)
    else:
        o_of_m = m                            # natural order

    wA = np.zeros((16, 128, 128), dtype=np.float32)
    for g3 in range(8):
        blk = M321[g3 * 128:(g3 + 1) * 128, g3 * 256:(g3 + 1) * 256]
        for k in range(2):
            wA[g3 * 2 + k] = blk[o_of_m, k * 128:(k + 1) * 128].T.astype(np.float32)

    bias_t = np.zeros((128, 4), dtype=np.float32)
    if rotated:
        # tile (Dp, P): p = qp*32 + j*2 + b, logical q = (qp - Dp) % 4
        # (col-group rotation so concurrent col-tiles use distinct groups);
        # u feature c4*64 + d4 with c4 = 2*(4P+q)+b, d4 = 16*Dp+j.
        wB = np.zeros((8, 128, 128), dtype=np.float32)
        p_idx = np.arange(128)
        qp, sp = p_idx // 32, p_idx % 32
        jj, bb = sp // 2, sp % 2
        for Dp in range(4):
            q = (qp - Dp) % 4
            d4 = 16 * Dp + jj
            for P in range(2):
                c4 = 2 * (4 * P + q) + bb
                for r in range(8):
                    wB[Dp * 2 + P, p_idx, r * 16 + jj] = T4[r * 64 + d4, c4 * 64 + d4]
        r_of_m, j_of_m = m // 16, m % 16
        for Dp in range(4):
            bias_t[:, Dp] = bias[r_of_m * 64 + 16 * Dp + j_of_m]
    else:
        # tile (h, g3): p = b*64 + d4 (u-tile g3 local feature),
        # m = a*64 + d4' (y features 128h + m); nonzero iff d4' == d4:
        # wB[p, m] = T4[(2h+a)*64 + d4, (2*g3+b)*64 + d4]
        wB = np.zeros((32, 128, 128), dtype=np.float32)
        d4 = np.arange(64)
        for h in range(4):
            for g3 in range(8):
                for a in range(2):
                    for b in range(2):
                        wB[h * 8 + g3, b * 64 + d4, a * 64 + d4] = \
                            T4[(2 * h + a) * 64 + d4, (2 * g3 + b) * 64 + d4]
        for h in range(4):
            bias_t[:, h] = bias[128 * h + m]
    return wA, wB, bias_t


# ------------------------------------------------------- v2 bass kernel
def _emit_v2(tc, nc, mybir, x, wA, wB, bt, y, repeats=1):
    """Rotated layout, PE col-group concurrency, slab DMA.

    Per (b, l-tile): stage A = 16 waves of 4 concurrent M=32 col-tiled
    matmuls (one per Dp, col group qp=(q+Dp)%4), stage B = 8 M=128 matmuls.
    x arrives f16 [B_SH, 512, 2048]; loaded once per (repeat, b) as 4
    row-wide slabs [128, 2048].  y written per (b, Dp) as one [128, 2045]
    DMA from a row-wide f32 SBUF tile.
    """
    import contextlib
    f16 = mybir.dt.float16
    f32 = mybir.dt.float32

    ctx = contextlib.ExitStack()
    with ctx:
        wpool = ctx.enter_context(tc.tile_pool(name="wpool", bufs=1))
        xpool = ctx.enter_context(tc.tile_pool(name="xpool", bufs=16))
        pspool = ctx.enter_context(tc.tile_pool(name="pspool", bufs=8,
                                                space="PSUM"))
        usb = ctx.enter_context(tc.tile_pool(name="usb", bufs=24))
        ywide = ctx.enter_context(tc.tile_pool(name="ywide", bufs=12))

        wA_sb = []
        for i in range(16):
            t = wpool.tile([128, 128], f16, tag=f"wA{i}", name=f"wA{i}")
            nc.scalar.dma_start(t[:], wA[i])
            wA_sb.append(t)
        wB_sb = []
        for i in range(8):
            t = wpool.tile([128, 128], f16, tag=f"wB{i}", name=f"wB{i}")
            nc.scalar.dma_start(t[:], wB[i])
            wB_sb.append(t)
        bt_sb = wpool.tile([128, 4], f32, tag="bt")
        nc.scalar.dma_start(bt_sb[:], bt[:])

        # y viewed as [b, r, Dp, j, l]: feature r*64 + 16*Dp + j lives at
        # partition m = r*16 + j of the (b, Dp) row-wide tile.
        yv = y.rearrange("b (r p j) l -> b r p j l", r=8, p=4, j=16)

        def stage_A_half(P, xs, l0, nt, u_sbuf):
            u_ps = [pspool.tile([128, NT], f32, tag="ps",
                                name=f"ups{P}_{d}") for d in range(4)]
            for q in range(4):
                g3 = 4 * P + q
                i_off = g3 // 2
                for k in range(2):
                    rhs = xs[(g3 % 2) * 2 + k][:, l0 + i_off:
                                               l0 + i_off + nt]
                    for Dp in range(4):
                        qp = (q + Dp) % 4
                        nc.tensor.matmul(
                            u_ps[Dp][qp * 32:(qp + 1) * 32, :nt],
                            wA_sb[g3 * 2 + k][:, Dp * 32:(Dp + 1) * 32],
                            rhs,
                            start=(k == 0), stop=(k == 1),
                            tile_position=(0, qp * 32),
                        )
            for Dp in range(4):
                t = usb.tile([128, NT], f16, tag="u_sb")
                # balance psum->sbuf copies: DVE 4/tile, ACT 4/tile
                on_act = Dp >= 2
                if on_act:
                    nc.scalar.copy(t[:, :nt], u_ps[Dp][:, :nt])
                else:
                    nc.vector.tensor_copy(t[:, :nt], u_ps[Dp][:, :nt])
                u_sbuf[(Dp, P)] = t

        def stage_B_half(dps, u_sbuf, y_tiles, b, l0, nt, last):
            for Dp in dps:
                y_ps = pspool.tile([128, NT], f32, tag="ps", name="yps")
                for P in range(2):
                    nc.tensor.matmul(
                        y_ps[:, :nt],
                        wB_sb[Dp * 2 + P][:],
                        u_sbuf[(Dp, P)][:, :nt],
                        start=(P == 0), stop=(P == 1),
                    )
                # y bias+copy split DVE/ACT so all four finish inside the
                # next tile's A-P0 window (psum ring slots for A-P1)
                if Dp < 2:
                    nc.vector.tensor_scalar_add(
                        y_tiles[Dp][:, l0:l0 + nt], y_ps[:, :nt],
                        bt_sb[:, Dp:Dp + 1])
                else:
                    nc.scalar.activation(
                        y_tiles[Dp][:, l0:l0 + nt], y_ps[:, :nt],
                        mybir.ActivationFunctionType.Identity,
                        bias=bt_sb[:, Dp:Dp + 1],
                    )
                if last:
                    # a store ring drives only 8 SDMA slots; split y stores
                    # across the sync and scalar rings
                    eng = nc.sync if Dp % 2 == 0 else nc.scalar
                    eng.dma_start(yv[b, :, Dp, :, 0:L_OUT],
                                  y_tiles[Dp][:, 0:L_OUT])

        # PE warm-up: ~2.6us of junk matmuls starting as soon as the kernel
        # launches, so the HAM un-throttles (1.2 -> 2.4 GHz) while the first
        # x slabs are still loading.  Emitted once; steady-state unaffected.
        warm = wpool.tile([128, NT], f16, tag="warm")
        nc.vector.memset(warm[:], 0)
        ps_w = pspool.tile([128, NT], f32, tag="ps", name="ps_warm")
        for _ in range(12):
            nc.tensor.matmul(ps_w[:, :NT], warm[:, :128], warm[:, :NT],
                             start=True, stop=True)

        pending = None
        for _rep in range(repeats):
            # load BOTH batches' slabs up front: a mid-repeat slab-issue
            # burst on sync stalls the batch-boundary tile ~0.8us
            xs_all = []
            for b in range(B_SH):
                xs = []
                for cb in range(4):
                    xt = xpool.tile([128, L_IN], f16, tag="xslab")
                    nc.sync.dma_start(xt[:],
                                      x[b, cb * 128:(cb + 1) * 128, :])
                    xs.append(xt)
                xs_all.append(xs)
            for b in range(B_SH):
                xs = xs_all[b]
                y_tiles = [ywide.tile([128, L_IN], f16, tag="ywide",
                                      name=f"yw{d}") for d in range(4)]
                for ti, (l0, nt) in enumerate(L_TILES):
                    u_sbuf = {}
                    stage_A_half(0, xs, l0, nt, u_sbuf)
                    stage_A_half(1, xs, l0, nt, u_sbuf)
                    if pending is not None:
                        stage_B_half((0, 1, 2, 3), *pending)
                    pending = (u_sbuf, y_tiles, b, l0, nt,
                               ti == len(L_TILES) - 1)
        stage_B_half((0, 1, 2, 3), *pending)


# ------------------------------------------------------------- bass kernel
def _emit(tc, nc, mybir, x, wA, wB, bt, y, mode, repeats=1):
    import contextlib
    f32 = mybir.dt.float32
    in_dt = {"f32r": mybir.dt.float32r, "f32": f32, "bf16": mybir.dt.bfloat16,
             "f16": mybir.dt.float16, "f16nat": mybir.dt.float16}[mode]
    rotated = mode in ("f32", "bf16", "f16")
    nB = 8 if rotated else 32
    # gpsimd DMA casts f32 -> 16-bit on the fly
    x_dma = nc.sync if mode in ("f32", "f32r") else nc.gpsimd

    ctx = contextlib.ExitStack()
    with ctx:
        wpool = ctx.enter_context(tc.tile_pool(name="wpool", bufs=1))
        xpool = ctx.enter_context(tc.tile_pool(name="xpool", bufs=12))
        upsum = ctx.enter_context(tc.tile_pool(name="upsum", bufs=4, space="PSUM"))
        ypsum = ctx.enter_context(tc.tile_pool(name="ypsum", bufs=4, space="PSUM"))
        usb = ctx.enter_context(tc.tile_pool(name="usb", bufs=24))
        ysb = ctx.enter_context(tc.tile_pool(name="ysb", bufs=12))

        wA_sb = []
        for i in range(16):
            t = wpool.tile([128, 128], in_dt, tag=f"wA{i}")
            nc.sync.dma_start(t[:], wA[i])
            wA_sb.append(t)
        wB_sb = []
        for i in range(nB):
            t = wpool.tile([128, 128], in_dt, tag=f"wB{i}")
            nc.sync.dma_start(t[:], wB[i])
            wB_sb.append(t)
        bt_sb = wpool.tile([128, 4], f32, tag="bt")
        nc.scalar.dma_start(bt_sb[:], bt[:])

        if rotated:
            # y viewed as [b, r, Dp, j, l]: feature r*64 + 16*Dp + j; DMA pairs
            # the (r, j) dims against the 128 SBUF partitions (m = r*16 + j).
            yv = y.rearrange("b (r p j) l -> b r p j l", r=8, p=4, j=16)

        def stage_A_rot(xs, l0, nt):
            u_sbuf = {}
            for P in range(2):
                u_ps = [upsum.tile([128, NT], f32, tag="u", name=f"ups{P}_{d}")
                        for d in range(4)]
                for qq in range(4):
                    g3 = 4 * P + qq
                    i_off = g3 // 2
                    for k in range(2):
                        rhs = xs[(g3 % 2) * 2 + k][:, i_off:i_off + nt]
                        for Dp in range(4):
                            qp = (qq + Dp) % 4
                            nc.tensor.matmul(
                                u_ps[Dp][qp * 32:(qp + 1) * 32, :nt],
                                wA_sb[g3 * 2 + k][:, Dp * 32:(Dp + 1) * 32],
                                rhs,
                                start=(k == 0), stop=(k == 1),
                                tile_position=(0, qp * 32),
                            )
                for Dp in range(4):
                    t = usb.tile([128, NT], in_dt, tag="u_sb")
                    nc.vector.tensor_copy(t[:, :nt], u_ps[Dp][:, :nt])
                    u_sbuf[(Dp, P)] = t
            return u_sbuf

        def stage_B_rot(u_sbuf, b, l0, nt):
            for Dp in range(4):
                y_ps = ypsum.tile([128, NT], f32, tag="y")
                for P in range(2):
                    nc.tensor.matmul(
                        y_ps[:, :nt],
                        wB_sb[Dp * 2 + P][:],
                        u_sbuf[(Dp, P)][:, :nt],
                        start=(P == 0), stop=(P == 1),
                    )
                t = ysb.tile([128, NT], f32, tag="y_sb")
                nc.scalar.activation(
                    t[:, :nt], y_ps[:, :nt],
                    mybir.ActivationFunctionType.Identity,
                    bias=bt_sb[:, Dp:Dp + 1],
                )
                nc.sync.dma_start(yv[b, :, Dp, :, l0:l0 + nt], t[:, :nt])

        def stage_A_nat(xs, l0, nt):
            u_sbuf = {}
            for half in range(2):
                u_ps = [upsum.tile([128, NT], f32, tag="u", name=f"ups{half}_{d}")
                        for d in range(4)]
                for gg in range(4):
                    g3 = 4 * half + gg
                    i_off = g3 // 2
                    for k in range(2):
                        rhs = xs[(g3 % 2) * 2 + k][:, i_off:i_off + nt]
                        nc.tensor.matmul(
                            u_ps[gg][:, :nt],
                            wA_sb[g3 * 2 + k][:],
                            rhs,
                            start=(k == 0), stop=(k == 1),
                        )
                for gg in range(4):
                    g3 = 4 * half + gg
                    t = usb.tile([128, NT], in_dt, tag="u_sb")
                    nc.vector.tensor_copy(t[:, :nt], u_ps[gg][:, :nt])
                    u_sbuf[g3] = t
            return u_sbuf

        def stage_B_nat(u_sbuf, b, l0, nt):
            for h in range(4):
                y_ps = ypsum.tile([128, NT], f32, tag="y")
                for g3 in range(8):
                    nc.tensor.matmul(
                        y_ps[:, :nt],
                        wB_sb[h * 8 + g3][:],
                        u_sbuf[g3][:, :nt],
                        start=(g3 == 0), stop=(g3 == 7),
                    )
                t = ysb.tile([128, NT], f32, tag="y_sb")
                nc.scalar.activation(
                    t[:, :nt], y_ps[:, :nt],
                    mybir.ActivationFunctionType.Identity,
                    bias=bt_sb[:, h:h + 1],
                )
                nc.sync.dma_start(y[b, h * 128:(h + 1) * 128, l0:l0 + nt],
                                  t[:, :nt])

        stage_A = stage_A_rot if rotated else stage_A_nat
        stage_B = stage_B_rot if rotated else stage_B_nat

        # software-pipelined emission: stage B of iteration t is emitted after
        # stage A of iteration t+1 so the PE never waits on u copies.
        # repeats > 1 re-emits the whole body (benchmarking only).
        # PE warm-up: ~2.6us of junk matmuls starting as soon as the kernel
        # launches, so the HAM un-throttles (1.2 -> 2.4 GHz) while the first
        # x slabs are still loading.  Emitted once; steady-state unaffected.
        warm = wpool.tile([128, NT], f16, tag="warm")
        nc.vector.memset(warm[:], 0)
        ps_w = pspool.tile([128, NT], f32, tag="ps", name="ps_warm")
        for _ in range(12):
            nc.tensor.matmul(ps_w[:, :NT], warm[:, :128], warm[:, :NT],
                             start=True, stop=True)

        pending = None
        for _rep in range(repeats):
            for b in range(B_SH):
                for (l0, nt) in L_TILES:
                    xs = []
                    for t4i in range(4):
                        # per-l-tile window [128, nt+3]: first matmuls start
                        # after ~260KB instead of a full 4MB batch load
                        xt = xpool.tile([128, NT + KW - 1], in_dt, tag="x")
                        x_dma.dma_start(
                            xt[:, :nt + KW - 1],
                            x[b, t4i * 128:(t4i + 1) * 128, l0:l0 + nt + KW - 1])
                        xs.append(xt)
                    u_sbuf = stage_A(xs, l0, nt)
                    if pending is not None:
                        stage_B(*pending)
                    pending = (u_sbuf, b, l0, nt)
        stage_B(*pending)


def _get_module(mode, repeats=1):
    key = (mode, repeats)
    if key in _CACHE:
        return _CACHE[key]
    import concourse.mybir as mybir
    import concourse.tile as tile
    from concourse import bacc

    nc = bacc.Bacc("TRN2", target_bir_lowering=False, debug=False,
                   enable_asserts=False, num_devices=N_CORES)
    f32 = mybir.dt.float32
    if mode == "v2":
        f16 = mybir.dt.float16
        x = nc.dram_tensor("x", [B_SH, C_IN, L_IN], f16,
                           kind="ExternalInput").ap()
        wA = nc.dram_tensor("wA", [16, 128, 128], f16,
                            kind="ExternalInput").ap()
        wB = nc.dram_tensor("wB", [8, 128, 128], f16,
                            kind="ExternalInput").ap()
        bt = nc.dram_tensor("bt", [128, 4], f32, kind="ExternalInput").ap()
        y = nc.dram_tensor("y", [B_SH, 512, L_OUT], f16,
                           kind="ExternalOutput").ap()
        with tile.TileContext(nc) as tc:
            _emit_v2(tc, nc, mybir, x, wA, wB, bt, y, repeats)
        nc.compile()
        _CACHE[key] = nc
        return nc
    w_dt = {"f32r": mybir.dt.float32r, "f32": f32, "bf16": mybir.dt.bfloat16,
            "f16": mybir.dt.float16, "f16nat": mybir.dt.float16}[mode]
    x_dt = f32 if mode in ("bf16", "f16", "f16nat") else w_dt
    nB = 8 if mode in ("f32", "bf16", "f16") else 32
    x = nc.dram_tensor("x", [B_SH, C_IN, L_IN], x_dt, kind="ExternalInput").ap()
    wA = nc.dram_tensor("wA", [16, 128, 128], w_dt, kind="ExternalInput").ap()
    wB = nc.dram_tensor("wB", [nB, 128, 128], w_dt, kind="ExternalInput").ap()
    bt = nc.dram_tensor("bt", [128, 4], f32, kind="ExternalInput").ap()
    y = nc.dram_tensor("y", [B_SH, 512, L_OUT], f32, kind="ExternalOutput").ap()

    with tile.TileContext(nc) as tc:
        _emit(tc, nc, mybir, x, wA, wB, bt, y, mode, repeats)
    nc.compile()
    _CACHE[key] = nc
    return nc


# ------------------------------------------------------------ entry point
def make_in_maps(inputs):
    """Host-side prep shared by kernel() and test.py's bench path."""
    x = np.ascontiguousarray(np.asarray(inputs["x"]), dtype=np.float32)
    twiddle = np.asarray(inputs["twiddle"], dtype=np.float32)
    bias = np.asarray(inputs["bias"], dtype=np.float32)

    rotated = MM_MODE in ("f32", "bf16", "f16", "v2")
    wA, wB, bt = _make_weights(twiddle, bias, rotated=rotated)
    if MM_MODE == "bf16":
        import ml_dtypes
        wA = wA.astype(ml_dtypes.bfloat16)
        wB = wB.astype(ml_dtypes.bfloat16)
    elif MM_MODE in ("f16", "f16nat", "v2"):
        wA = wA.astype(np.float16)
        wB = wB.astype(np.float16)
    if MM_MODE == "v2":
        x = np.ascontiguousarray(x.astype(np.float16))
    return [
        {"x": x[c * B_SH:(c + 1) * B_SH], "wA": wA, "wB": wB, "bt": bt}
        for c in range(N_CORES)
    ]


def kernel(x, twiddle, bias):
    global LAST_RESULT
    from concourse import bass_utils

    in_maps = make_in_maps({"x": x, "twiddle": twiddle, "bias": bias})
    nc = _get_module(MM_MODE)
    res = bass_utils.run_bass_kernel_spmd(nc, in_maps, list(range(N_CORES)))
    LAST_RESULT = res
    out = np.concatenate([res.results[c]["y"] for c in range(N_CORES)], axis=0)
    return out.astype(np.float32)



# revision 38
# speedup vs baseline: 1.0073x; 1.0073x over previous
"""DeBut-factorized 1D conv (kernel_size=4) on 8 Trainium2 NeuronCores.

Math: y[b,:,l] = W @ im2col_row(b,l) + bias, where W (512x2048) is a chain of
4 block-diagonal butterfly factors T4@T3@T2@T1.  We fold T3@T2@T1 into stage A
(block-diagonal, 8 dense [128 out x 256 in] blocks) and keep T4 (an 8x16 grid
of 64-wide diagonals) as stage B.

Default mode "v2" (rotated layout, ~45us/exec steady state, ~6.7e-4 rel err):
  * stage A: per 512-column l-tile, 16 waves of 4 CONCURRENT M=32 matmuls on
    the four PE column groups (tile_position=(0, qp*32), qp=(q+Dp)%4 rotation
    so each wave's tiles hit distinct groups).  A wave costs one N=512 stream
    (~216ns) - the col-tiled matmuls run concurrently (measured dstart~10ns).
    The M=32 splitting lands u in a (Dp, P) d-major-tile layout...
  * ...which collapses stage B (T4) to 8 M=128 matmuls per tile: y-tile Dp
    contracts just u tiles (Dp, P=0..1).  24 wave-equivalents/tile total vs
    48 for the natural layout: PE floor 24*512cy = 41.5us/core @2.4GHz.
  * im2col is free: stage-A rhs for (window offset i, channel block) is a
    shifted free-dim slice of a resident [128, 2048] x slab.
  * x is cast to fp16 on the host (halves load bytes; 4KB descriptors); y is
    stored as fp16 and upcast on the host (stores only run on 8 of 16 SDMA
    engines, so fp32 stores at 8.4MB/core would floor at ~47us > PE floor).
  * psum->sbuf traffic balanced DVE/ACT (4+2 ops each per tile); y stores
    split across the sync and scalar DGE rings; PE warm-up burst unthrottles
    the HAM clock gate during the initial slab loads.

Sharding: data-parallel over batch - each of the 8 cores takes 2 of the 16
batches; twiddle/bias-derived weights are replicated (host-precomputed).
"""

import os
import numpy as np

# ---------------------------------------------------------------- constants
R_SHAPES = [[2048, 2048, 4, 4, 1],
            [2048, 2048, 8, 8, 4],
            [1024, 2048, 4, 8, 32],
            [512, 1024, 8, 16, 64]]
KW = 4              # conv kernel size
B_FULL, C_IN, L_IN = 16, 512, 2048
L_OUT = L_IN - (KW - 1)          # 2045
N_CORES = 8
B_SH = B_FULL // N_CORES         # 2 batches per core
NT = 512                         # l-tile width (PSUM bank = 512 f32)
# last tile overlaps by 3 columns (recomputed identically) so every tile is a
# uniform 512 wide — fp32r matmuls require an even free dim, and 512 keeps the
# window reads inside x's 2048 columns.
L_TILES = [(0, 512), (512, 512), (1024, 512), (1533, 512)]

# "v2":  rotated layout exploiting PE col-group concurrency (16 4-wide
#        stage-A waves + 8 stage-B matmuls per tile = 24 wave-equivalents
#        vs natural's 48), x host-cast to f16 + full-row slab DMA loads,
#        y accumulated in row-wide SBUF tiles for 8KB-descriptor stores.
# "f16nat": fp16 inputs, fp32 accumulate, natural layout (~83us measured)
# "f32r": reduced-precision full-rate fp32 PE path (natural layout, ~3e-4)
# "f32" : exact fp32, 4 cyc/row (rotated layout)
# "bf16"/"f16": 16-bit rotated col-tiled layouts (per-l-tile x DMA; slower)
MM_MODE = os.environ.get("DEBUT_MM_MODE", "v2")

_CACHE = {}          # mode -> compiled Bacc module
LAST_RESULT = None   # BassKernelResults of the most recent run (for test.py)


# ------------------------------------------------------- host-side weights
def _build_T_matrices(twiddle):
    Ts, p = [], 0
    for (osz, isz, row, col, diag) in R_SHAPES:
        g = isz // (col * diag)
        n_p = col * osz
        t = twiddle[p:p + n_p].reshape(g, diag, row, col).transpose(0, 2, 3, 1)
        T = np.zeros((osz, isz), dtype=np.float64)
        gi = np.arange(g)[:, None, None, None]
        ri = np.arange(row)[None, :, None, None]
        ci = np.arange(col)[None, None, :, None]
        di = np.arange(diag)[None, None, None, :]
        oi = np.broadcast_to((gi * row + ri) * diag + di, t.shape)
        ii = np.broadcast_to((gi * col + ci) * diag + di, t.shape)
        T[oi.ravel(), ii.ravel()] = t.ravel()
        Ts.append(T)
        p += n_p
    return Ts


def _make_weights(twiddle, bias, rotated):
    """Returns wA [16,128,128], wB list, bias_t [128,4] (f32).

    natural: wB has 32 tiles (h*8 + g3), y-tile h = features [128h, 128h+128).
    rotated: wB has 8 tiles (Dp*2 + P), y-tile Dp features r*64 + 16*Dp + j
             at partition m = r*16 + j.
    """
    T1, T2, T3, T4 = _build_T_matrices(twiddle.astype(np.float64))
    M321 = T3 @ T2 @ T1                       # block-diag: 8 x [128 out, 256 in]

    m = np.arange(128)
    if rotated:
        # stage-A M column m = Dp*32 + j*2 + b -> block-local feature b*64+16Dp+j
        Dp_of_m, s = m // 32, m % 32
        o_of_m = (s % 2) * 64 + 16 * Dp_of_m + (s // 2)
    else:
        o_of_m = m                            # natural order

    wA = np.zeros((16, 128, 128), dtype=np.float32)
    for g3 in range(8):
        blk = M321[g3 * 128:(g3 + 1) * 128, g3 * 256:(g3 + 1) * 256]
        for k in range(2):
            wA[g3 * 2 + k] = blk[o_of_m, k * 128:(k + 1) * 128].T.astype(np.float32)

    bias_t = np.zeros((128, 4), dtype=np.float32)
    if rotated:
        # tile (Dp, P): p = qp*32 + j*2 + b, logical q = (qp - Dp) % 4
        # (col-group rotation so concurrent col-tiles use distinct groups);
        # u feature c4*64 + d4 with c4 = 2*(4P+q)+b, d4 = 16*Dp+j.
        wB = np.zeros((8, 128, 128), dtype=np.float32)
        p_idx = np.arange(128)
        qp, sp = p_idx // 32, p_idx % 32
        jj, bb = sp // 2, sp % 2
        for Dp in range(4):
            q = (qp - Dp) % 4
            d4 = 16 * Dp + jj
            for P in range(2):
                c4 = 2 * (4 * P + q) + bb
                for r in range(8):
                    wB[Dp * 2 + P, p_idx, r * 16 + jj] = T4[r * 64 + d4, c4 * 64 + d4]
        r_of_m, j_of_m = m // 16, m % 16
        for Dp in range(4):
            bias_t[:, Dp] = bias[r_of_m * 64 + 16 * Dp + j_of_m]
    else:
        # tile (h, g3): p = b*64 + d4 (u-tile g3 local feature),
        # m = a*64 + d4' (y features 128h + m); nonzero iff d4' == d4:
        # wB[p, m] = T4[(2h+a)*64 + d4, (2*g3+b)*64 + d4]
        wB = np.zeros((32, 128, 128), dtype=np.float32)
        d4 = np.arange(64)
        for h in range(4):
            for g3 in range(8):
                for a in range(2):
                    for b in range(2):
                        wB[h * 8 + g3, b * 64 + d4, a * 64 + d4] = \
                            T4[(2 * h + a) * 64 + d4, (2 * g3 + b) * 64 + d4]
        for h in range(4):
            bias_t[:, h] = bias[128 * h + m]
    return wA, wB, bias_t


# ------------------------------------------------------- v2 bass kernel
def _emit_v2(tc, nc, mybir, x, wA, wB, bt, y, repeats=1):
    """Rotated layout, PE col-group concurrency, slab DMA.

    Per (b, l-tile): stage A = 16 waves of 4 concurrent M=32 col-tiled
    matmuls (one per Dp, col group qp=(q+Dp)%4), stage B = 8 M=128 matmuls.
    x arrives f16 [B_SH, 512, 2048]; loaded once per (repeat, b) as 4
    row-wide slabs [128, 2048].  y written per (b, Dp) as one [128, 2045]
    DMA from a row-wide f32 SBUF tile.
    """
    import contextlib
    f16 = mybir.dt.float16
    f32 = mybir.dt.float32

    ctx = contextlib.ExitStack()
    with ctx:
        wpool = ctx.enter_context(tc.tile_pool(name="wpool", bufs=1))
        xpool = ctx.enter_context(tc.tile_pool(name="xpool", bufs=10))
        pspool = ctx.enter_context(tc.tile_pool(name="pspool", bufs=8,
                                                space="PSUM"))
        usb = ctx.enter_context(tc.tile_pool(name="usb", bufs=24))
        ywide = ctx.enter_context(tc.tile_pool(name="ywide", bufs=12))

        wA_sb = []
        for i in range(16):
            t = wpool.tile([128, 128], f16, tag=f"wA{i}", name=f"wA{i}")
            nc.scalar.dma_start(t[:], wA[i])
            wA_sb.append(t)
        wB_sb = []
        for i in range(8):
            t = wpool.tile([128, 128], f16, tag=f"wB{i}", name=f"wB{i}")
            nc.scalar.dma_start(t[:], wB[i])
            wB_sb.append(t)
        bt_sb = wpool.tile([128, 4], f32, tag="bt")
        nc.scalar.dma_start(bt_sb[:], bt[:])

        # y viewed as [b, r, Dp, j, l]: feature r*64 + 16*Dp + j lives at
        # partition m = r*16 + j of the (b, Dp) row-wide tile.
        yv = y.rearrange("b (r p j) l -> b r p j l", r=8, p=4, j=16)

        def stage_A_half(P, xs, l0, nt, u_sbuf):
            u_ps = [pspool.tile([128, NT], f32, tag="ps",
                                name=f"ups{P}_{d}") for d in range(4)]
            for q in range(4):
                g3 = 4 * P + q
                i_off = g3 // 2
                for k in range(2):
                    rhs = xs[(g3 % 2) * 2 + k][:, l0 + i_off:
                                               l0 + i_off + nt]
                    for Dp in range(4):
                        qp = (q + Dp) % 4
                        nc.tensor.matmul(
                            u_ps[Dp][qp * 32:(qp + 1) * 32, :nt],
                            wA_sb[g3 * 2 + k][:, Dp * 32:(Dp + 1) * 32],
                            rhs,
                            start=(k == 0), stop=(k == 1),
                            tile_position=(0, qp * 32),
                        )
            for Dp in range(4):
                t = usb.tile([128, NT], f16, tag="u_sb")
                # balance psum->sbuf copies: DVE 4/tile, ACT 4/tile
                on_act = Dp >= 2
                if on_act:
                    nc.scalar.copy(t[:, :nt], u_ps[Dp][:, :nt])
                else:
                    nc.vector.tensor_copy(t[:, :nt], u_ps[Dp][:, :nt])
                u_sbuf[(Dp, P)] = t

        def stage_B_half(dps, u_sbuf, y_tiles, b, l0, nt, last):
            for Dp in dps:
                y_ps = pspool.tile([128, NT], f32, tag="ps", name="yps")
                for P in range(2):
                    nc.tensor.matmul(
                        y_ps[:, :nt],
                        wB_sb[Dp * 2 + P][:],
                        u_sbuf[(Dp, P)][:, :nt],
                        start=(P == 0), stop=(P == 1),
                    )
                # y bias+copy split DVE/ACT so all four finish inside the
                # next tile's A-P0 window (psum ring slots for A-P1)
                if Dp < 2:
                    nc.vector.tensor_scalar_add(
                        y_tiles[Dp][:, l0:l0 + nt], y_ps[:, :nt],
                        bt_sb[:, Dp:Dp + 1])
                else:
                    nc.scalar.activation(
                        y_tiles[Dp][:, l0:l0 + nt], y_ps[:, :nt],
                        mybir.ActivationFunctionType.Identity,
                        bias=bt_sb[:, Dp:Dp + 1],
                    )
                if last:
                    # a store ring drives only 8 SDMA slots; split y stores
                    # across the sync and scalar rings
                    eng = nc.sync if Dp % 2 == 0 else nc.scalar
                    eng.dma_start(yv[b, :, Dp, :, 0:L_OUT],
                                  y_tiles[Dp][:, 0:L_OUT])

        # PE warm-up: ~2.6us of junk matmuls starting as soon as the kernel
        # launches, so the HAM un-throttles (1.2 -> 2.4 GHz) while the first
        # x slabs are still loading.  Emitted once; steady-state unaffected.
        warm = wpool.tile([128, NT], f16, tag="warm")
        nc.vector.memset(warm[:], 0)
        ps_w = pspool.tile([128, NT], f32, tag="ps", name="ps_warm")
        for _ in range(12):
            nc.tensor.matmul(ps_w[:, :NT], warm[:, :128], warm[:, :NT],
                             start=True, stop=True)

        pending = None
        for _rep in range(repeats):
            for b in range(B_SH):
                xs = []
                for cb in range(4):
                    xt = xpool.tile([128, L_IN], f16, tag="xslab")
                    nc.sync.dma_start(xt[:],
                                      x[b, cb * 128:(cb + 1) * 128, :])
                    xs.append(xt)
                y_tiles = [ywide.tile([128, L_IN], f16, tag="ywide",
                                      name=f"yw{d}") for d in range(4)]
                for ti, (l0, nt) in enumerate(L_TILES):
                    u_sbuf = {}
                    stage_A_half(0, xs, l0, nt, u_sbuf)
                    stage_A_half(1, xs, l0, nt, u_sbuf)
                    if pending is not None:
                        stage_B_half((0, 1, 2, 3), *pending)
                    pending = (u_sbuf, y_tiles, b, l0, nt,
                               ti == len(L_TILES) - 1)
        stage_B_half((0, 1, 2, 3), *pending)


# ------------------------------------------------------------- bass kernel
def _emit(tc, nc, mybir, x, wA, wB, bt, y, mode, repeats=1):
    import contextlib
    f32 = mybir.dt.float32
    in_dt = {"f32r": mybir.dt.float32r, "f32": f32, "bf16": mybir.dt.bfloat16,
             "f16": mybir.dt.float16, "f16nat": mybir.dt.float16}[mode]
    rotated = mode in ("f32", "bf16", "f16")
    nB = 8 if rotated else 32
    # gpsimd DMA casts f32 -> 16-bit on the fly
    x_dma = nc.sync if mode in ("f32", "f32r") else nc.gpsimd

    ctx = contextlib.ExitStack()
    with ctx:
        wpool = ctx.enter_context(tc.tile_pool(name="wpool", bufs=1))
        xpool = ctx.enter_context(tc.tile_pool(name="xpool", bufs=12))
        upsum = ctx.enter_context(tc.tile_pool(name="upsum", bufs=4, space="PSUM"))
        ypsum = ctx.enter_context(tc.tile_pool(name="ypsum", bufs=4, space="PSUM"))
        usb = ctx.enter_context(tc.tile_pool(name="usb", bufs=24))
        ysb = ctx.enter_context(tc.tile_pool(name="ysb", bufs=12))

        wA_sb = []
        for i in range(16):
            t = wpool.tile([128, 128], in_dt, tag=f"wA{i}")
            nc.sync.dma_start(t[:], wA[i])
            wA_sb.append(t)
        wB_sb = []
        for i in range(nB):
            t = wpool.tile([128, 128], in_dt, tag=f"wB{i}")
            nc.sync.dma_start(t[:], wB[i])
            wB_sb.append(t)
        bt_sb = wpool.tile([128, 4], f32, tag="bt")
        nc.scalar.dma_start(bt_sb[:], bt[:])

        if rotated:
            # y viewed as [b, r, Dp, j, l]: feature r*64 + 16*Dp + j; DMA pairs
            # the (r, j) dims against the 128 SBUF partitions (m = r*16 + j).
            yv = y.rearrange("b (r p j) l -> b r p j l", r=8, p=4, j=16)

        def stage_A_rot(xs, l0, nt):
            u_sbuf = {}
            for P in range(2):
                u_ps = [upsum.tile([128, NT], f32, tag="u", name=f"ups{P}_{d}")
                        for d in range(4)]
                for qq in range(4):
                    g3 = 4 * P + qq
                    i_off = g3 // 2
                    for k in range(2):
                        rhs = xs[(g3 % 2) * 2 + k][:, i_off:i_off + nt]
                        for Dp in range(4):
                            qp = (qq + Dp) % 4
                            nc.tensor.matmul(
                                u_ps[Dp][qp * 32:(qp + 1) * 32, :nt],
                                wA_sb[g3 * 2 + k][:, Dp * 32:(Dp + 1) * 32],
                                rhs,
                                start=(k == 0), stop=(k == 1),
                                tile_position=(0, qp * 32),
                            )
                for Dp in range(4):
                    t = usb.tile([128, NT], in_dt, tag="u_sb")
                    nc.vector.tensor_copy(t[:, :nt], u_ps[Dp][:, :nt])
                    u_sbuf[(Dp, P)] = t
            return u_sbuf

        def stage_B_rot(u_sbuf, b, l0, nt):
            for Dp in range(4):
                y_ps = ypsum.tile([128, NT], f32, tag="y")
                for P in range(2):
                    nc.tensor.matmul(
                        y_ps[:, :nt],
                        wB_sb[Dp * 2 + P][:],
                        u_sbuf[(Dp, P)][:, :nt],
                        start=(P == 0), stop=(P == 1),
                    )
                t = ysb.tile([128, NT], f32, tag="y_sb")
                nc.scalar.activation(
                    t[:, :nt], y_ps[:, :nt],
                    mybir.ActivationFunctionType.Identity,
                    bias=bt_sb[:, Dp:Dp + 1],
                )
                nc.sync.dma_start(yv[b, :, Dp, :, l0:l0 + nt], t[:, :nt])

        def stage_A_nat(xs, l0, nt):
            u_sbuf = {}
            for half in range(2):
                u_ps = [upsum.tile([128, NT], f32, tag="u", name=f"ups{half}_{d}")
                        for d in range(4)]
                for gg in range(4):
                    g3 = 4 * half + gg
                    i_off = g3 // 2
                    for k in range(2):
                        rhs = xs[(g3 % 2) * 2 + k][:, i_off:i_off + nt]
                        nc.tensor.matmul(
                            u_ps[gg][:, :nt],
                            wA_sb[g3 * 2 + k][:],
                            rhs,
                            start=(k == 0), stop=(k == 1),
                        )
                for gg in range(4):
                    g3 = 4 * half + gg
                    t = usb.tile([128, NT], in_dt, tag="u_sb")
                    nc.vector.tensor_copy(t[:, :nt], u_ps[gg][:, :nt])
                    u_sbuf[g3] = t
            return u_sbuf

        def stage_B_nat(u_sbuf, b, l0, nt):
            for h in range(4):
                y_ps = ypsum.tile([128, NT], f32, tag="y")
                for g3 in range(8):
                    nc.tensor.matmul(
                        y_ps[:, :nt],
                        wB_sb[h * 8 + g3][:],
                        u_sbuf[g3][:, :nt],
                        start=(g3 == 0), stop=(g3 == 7),
                    )
                t = ysb.tile([128, NT], f32, tag="y_sb")
                nc.scalar.activation(
                    t[:, :nt], y_ps[:, :nt],
                    mybir.ActivationFunctionType.Identity,
                    bias=bt_sb[:, h:h + 1],
                )
                nc.sync.dma_start(y[b, h * 128:(h + 1) * 128, l0:l0 + nt],
                                  t[:, :nt])

        stage_A = stage_A_rot if rotated else stage_A_nat
        stage_B = stage_B_rot if rotated else stage_B_nat

        # software-pipelined emission: stage B of iteration t is emitted after
        # stage A of iteration t+1 so the PE never waits on u copies.
        # repeats > 1 re-emits the whole body (benchmarking only).
        # PE warm-up: ~2.6us of junk matmuls starting as soon as the kernel
        # launches, so the HAM un-throttles (1.2 -> 2.4 GHz) while the first
        # x slabs are still loading.  Emitted once; steady-state unaffected.
        warm = wpool.tile([128, NT], f16, tag="warm")
        nc.vector.memset(warm[:], 0)
        ps_w = pspool.tile([128, NT], f32, tag="ps", name="ps_warm")
        for _ in range(12):
            nc.tensor.matmul(ps_w[:, :NT], warm[:, :128], warm[:, :NT],
                             start=True, stop=True)

        pending = None
        for _rep in range(repeats):
            for b in range(B_SH):
                for (l0, nt) in L_TILES:
                    xs = []
                    for t4i in range(4):
                        # per-l-tile window [128, nt+3]: first matmuls start
                        # after ~260KB instead of a full 4MB batch load
                        xt = xpool.tile([128, NT + KW - 1], in_dt, tag="x")
                        x_dma.dma_start(
                            xt[:, :nt + KW - 1],
                            x[b, t4i * 128:(t4i + 1) * 128, l0:l0 + nt + KW - 1])
                        xs.append(xt)
                    u_sbuf = stage_A(xs, l0, nt)
                    if pending is not None:
                        stage_B(*pending)
                    pending = (u_sbuf, b, l0, nt)
        stage_B(*pending)


def _get_module(mode, repeats=1):
    key = (mode, repeats)
    if key in _CACHE:
        return _CACHE[key]
    import concourse.mybir as mybir
    import concourse.tile as tile
    from concourse import bacc

    nc = bacc.Bacc("TRN2", target_bir_lowering=False, debug=False,
                   enable_asserts=False, num_devices=N_CORES)
    f32 = mybir.dt.float32
    if mode == "v2":
        f16 = mybir.dt.float16
        x = nc.dram_tensor("x", [B_SH, C_IN, L_IN], f16,
                           kind="ExternalInput").ap()
        wA = nc.dram_tensor("wA", [16, 128, 128], f16,
                            kind="ExternalInput").ap()
        wB = nc.dram_tensor("wB", [8, 128, 128], f16,
                            kind="ExternalInput").ap()
        bt = nc.dram_tensor("bt", [128, 4], f32, kind="ExternalInput").ap()
        y = nc.dram_tensor("y", [B_SH, 512, L_OUT], f16,
                           kind="ExternalOutput").ap()
        with tile.TileContext(nc) as tc:
            _emit_v2(tc, nc, mybir, x, wA, wB, bt, y, repeats)
        nc.compile()
        _CACHE[key] = nc
        return nc
    w_dt = {"f32r": mybir.dt.float32r, "f32": f32, "bf16": mybir.dt.bfloat16,
            "f16": mybir.dt.float16, "f16nat": mybir.dt.float16}[mode]
    x_dt = f32 if mode in ("bf16", "f16", "f16nat") else w_dt
    nB = 8 if mode in ("f32", "bf16", "f16") else 32
    x = nc.dram_tensor("x", [B_SH, C_IN, L_IN], x_dt, kind="ExternalInput").ap()
    wA = nc.dram_tensor("wA", [16, 128, 128], w_dt, kind="ExternalInput").ap()
    wB = nc.dram_tensor("wB", [nB, 128, 128], w_dt, kind="ExternalInput").ap()
    bt = nc.dram_tensor("bt", [128, 4], f32, kind="ExternalInput").ap()
    y = nc.dram_tensor("y", [B_SH, 512, L_OUT], f32, kind="ExternalOutput").ap()

    with tile.TileContext(nc) as tc:
        _emit(tc, nc, mybir, x, wA, wB, bt, y, mode, repeats)
    nc.compile()
    _CACHE[key] = nc
    return nc


# ------------------------------------------------------------ entry point
def make_in_maps(inputs):
    """Host-side prep shared by kernel() and test.py's bench path."""
    x = np.ascontiguousarray(np.asarray(inputs["x"]), dtype=np.float32)
    twiddle = np.asarray(inputs["twiddle"], dtype=np.float32)
    bias = np.asarray(inputs["bias"], dtype=np.float32)

    rotated = MM_MODE in ("f32", "bf16", "f16", "v2")
    wA, wB, bt = _make_weights(twiddle, bias, rotated=rotated)
    if MM_MODE == "bf16":
        import ml_dtypes
        wA = wA.astype(ml_dtypes.bfloat16)
        wB = wB.astype(ml_dtypes.bfloat16)
    elif MM_MODE in ("f16", "f16nat", "v2"):
        wA = wA.astype(np.float16)
        wB = wB.astype(np.float16)
    if MM_MODE == "v2":
        x = np.ascontiguousarray(x.astype(np.float16))
    return [
        {"x": x[c * B_SH:(c + 1) * B_SH], "wA": wA, "wB": wB, "bt": bt}
        for c in range(N_CORES)
    ]


def kernel(x, twiddle, bias):
    global LAST_RESULT
    from concourse import bass_utils

    in_maps = make_in_maps({"x": x, "twiddle": twiddle, "bias": bias})
    nc = _get_module(MM_MODE)
    res = bass_utils.run_bass_kernel_spmd(nc, in_maps, list(range(N_CORES)))
    LAST_RESULT = res
    out = np.concatenate([res.results[c]["y"] for c in range(N_CORES)], axis=0)
    return out.astype(np.float32)

